# revision 4
# baseline (speedup 1.0000x reference)
"""3x3 median blur (zero padding) on (16, 3, 512, 512) f32 for 8 NeuronCores.

Sharding: pure data parallel — batch 16 -> 2 per core; each core's 6 images
(2b x 3c) are processed as 3 two-image tiles.

Per-pair layout: [128, 10, 514] — 2 images x 512 rows = 1024 rows, 8 per
partition (image A on partitions 0..63, image B on 64..127), plus 1 halo row
above (tile row 0) and below (row 9); width padded by 1 zero column each
side so all 3x3 taps are free-dim offsets.

Pipeline per pair:
  - DMA fp32 central rows + halo rows (two chunks, each converted to bf16 on
    the scalar engine as soon as it lands, overlapping the other chunk's DMA).
  - DVE vertical sort3 per column (shared pair min/max amortizes ops):
    s0/s1/s2 = col min/med/max. All tensor_tensor ops in bf16: unit-stride
    16-bit operands run in the DVE's 2x perf mode (fp32 TT is 1x).
  - DVE horizontal: A=max3(s0), C=min3(s2), B=med3(s1) via sliding shifts,
    out = med3(A,B,C).
  - Scalar engine converts bf16 -> fp32 and stores via its own HWDGE ring
    (keeps stores out of the load ring), in two chunks to shorten the drain.

bf16 rounding gives rel err ~2^-9 (measured 1.7e-3), far under the 2e-2
gate; the median picks a window element so no error accumulation occurs.
"""

import numpy as np

B, C, H, W = 16, 3, 512, 512
N_CORES = 8
B_LOC = B // N_CORES          # 2 batches per core
IMGS = B_LOC * C              # 6 images per core
PAIRS = IMGS // 2             # 3 image pairs per core
RP = 8                        # output rows per partition (128*8 = 2 images)
WP = W + 2                    # padded row width

_STATE = {}


def _mk_ap(base_ap, offset, pattern):
    """Clone an AP with a manual [step, count] pattern (element units)."""
    import concourse.mybir as mybir

    ap = base_ap.copy()
    ap.ap = mybir.VecI64Pair(pattern)
    ap.offset = offset
    return ap


def _build_nc(repeat=0):
    import contextlib

    import concourse.bacc as bacc
    import concourse.mybir as mybir
    from concourse.tile import TileContext

    f32 = mybir.dt.float32
    bf16 = mybir.dt.bfloat16
    Alu = mybir.AluOpType

    nc = bacc.Bacc("TRN2")
    x = nc.dram_tensor("x", [PAIRS, 2 * H, W], f32, kind="ExternalInput")
    y = nc.dram_tensor("y", [PAIRS, 2 * H, W], f32, kind="ExternalOutput")

    with TileContext(nc) as tc:
        with (
            tc.tile_pool(name="io", bufs=2) as io,
            tc.tile_pool(name="cmp", bufs=1) as cmp_,
            tc.For_i(0, repeat, 1) if repeat else contextlib.nullcontext(),
        ):
            for pi in range(PAIRS):
                xi = x[pi]
                yi = y[pi]

                # ---- load fp32 [128, 10, 514]: tile row r+1 = pair row 8p+r
                tf = io.tile([128, 10, WP], f32, name="tf", tag="tf")
                # zero pad columns + halo rows (halo DMAs below leave the
                # image-boundary partitions' rows 0/9 zero)
                nc.gpsimd.memset(tf[:, :, 0 : WP : WP - 1], 0.0)
                nc.gpsimd.memset(tf[:, 0:10:9, 1 : W + 1], 0.0)
                xr = xi.rearrange("(p r) w -> p r w", p=128)
                tb = cmp_.tile([128, 10, WP], bf16, name="tb", tag="tb")
                # chunk 1: tile rows 1..4 + halo above
                nc.sync.dma_start(out=tf[:, 1:5, 1 : W + 1], in_=xr[:, 0:4, :])
                # halo above: pair row 8p-1 -> tile row 0 (skip p=0 and p=64,
                # which stay zero = image top padding)
                nc.sync.dma_start(out=tf[1:64, 0, 1 : W + 1], in_=xi[7:505:8, :])
                nc.sync.dma_start(
                    out=tf[65:128, 0, 1 : W + 1], in_=xi[519:1017:8, :]
                )
                nc.scalar.copy(out=tb[:, 0:5, :], in_=tf[:, 0:5, :])
                # chunk 2: tile rows 5..8 + halo below (skip p=63 and p=127)
                nc.sync.dma_start(out=tf[:, 5:9, 1 : W + 1], in_=xr[:, 4:8, :])
                nc.sync.dma_start(out=tf[0:63, 9, 1 : W + 1], in_=xi[8:505:8, :])
                nc.sync.dma_start(
                    out=tf[64:127, 9, 1 : W + 1], in_=xi[520:1017:8, :]
                )
                nc.scalar.copy(out=tb[:, 5:10, :], in_=tf[:, 5:10, :])

                # ---- vertical sort3 (DVE bf16 2x). Pair min/max of rows
                # (1,2),(3,4),(5,6),(7,8); output row j=2p+r combines pair p
                # with c-row 2p+3r.
                pvmin = cmp_.tile([128, 4, WP], bf16, name="pvmin", tag="pvmin")
                pvmax = cmp_.tile([128, 4, WP], bf16, name="pvmax", tag="pvmax")
                nc.vector.tensor_tensor(
                    out=pvmin[:], in0=tb[:, 1:9:2, :], in1=tb[:, 2:10:2, :],
                    op=Alu.min)
                nc.vector.tensor_tensor(
                    out=pvmax[:], in0=tb[:, 1:9:2, :], in1=tb[:, 2:10:2, :],
                    op=Alu.max)

                s0 = cmp_.tile([128, RP, WP], bf16, name="s0", tag="s0")
                s1 = cmp_.tile([128, RP, WP], bf16, name="s1", tag="s1")
                s2 = cmp_.tile([128, RP, WP], bf16, name="s2", tag="s2")
                tq = cmp_.tile([128, RP, WP], bf16, name="tq", tag="tq")
                c_ap = _mk_ap(
                    tb[:], 0, [[10 * WP, 128], [2 * WP, 4], [3 * WP, 2], [1, WP]]
                )
                pvmin_b = _mk_ap(
                    pvmin[:], 0, [[4 * WP, 128], [WP, 4], [0, 2], [1, WP]]
                )
                pvmax_b = _mk_ap(
                    pvmax[:], 0, [[4 * WP, 128], [WP, 4], [0, 2], [1, WP]]
                )

                def s_ap(tile):
                    return _mk_ap(
                        tile[:], 0,
                        [[RP * WP, 128], [2 * WP, 4], [WP, 2], [1, WP]],
                    )

                nc.vector.tensor_tensor(
                    out=s_ap(s0), in0=pvmin_b, in1=c_ap, op=Alu.min)
                nc.vector.tensor_tensor(
                    out=s_ap(s2), in0=pvmax_b, in1=c_ap, op=Alu.max)
                nc.vector.tensor_tensor(
                    out=s_ap(tq), in0=pvmax_b, in1=c_ap, op=Alu.min)
                nc.vector.tensor_tensor(
                    out=s_ap(s1), in0=pvmin_b, in1=s_ap(tq), op=Alu.max)

                # ---- horizontal (DVE bf16 2x): A=max3(s0), C=min3(s2),
                # B=med3(s1), out=med3(A,B,C). Width-padded tiles keep all
                # row starts 4B-aligned.
                a1 = cmp_.tile([128, RP, WP], bf16, name="a1", tag="a1")
                c1 = cmp_.tile([128, RP, WP], bf16, name="c1", tag="c1")
                m1 = cmp_.tile([128, RP, WP], bf16, name="m1", tag="m1")
                M1 = cmp_.tile([128, RP, WP], bf16, name="M1", tag="M1")
                TT = nc.vector.tensor_tensor
                TT(out=a1[:, :, 0:513], in0=s0[:, :, 0:513],
                   in1=s0[:, :, 1:514], op=Alu.max)
                TT(out=c1[:, :, 0:513], in0=s2[:, :, 0:513],
                   in1=s2[:, :, 1:514], op=Alu.min)
                TT(out=m1[:, :, 0:513], in0=s1[:, :, 0:513],
                   in1=s1[:, :, 1:514], op=Alu.min)
                TT(out=M1[:, :, 0:513], in0=s1[:, :, 0:513],
                   in1=s1[:, :, 1:514], op=Alu.max)
                # tails: A -> a1 in place; C -> c1; t=min(M1,s1+2) -> M1;
                # B=max(m1,t) -> M1
                TT(out=a1[:, :, 0:512], in0=a1[:, :, 0:512],
                   in1=s0[:, :, 2:514], op=Alu.max)
                TT(out=c1[:, :, 0:512], in0=c1[:, :, 0:512],
                   in1=s2[:, :, 2:514], op=Alu.min)
                TT(out=M1[:, :, 0:512], in0=M1[:, :, 0:512],
                   in1=s1[:, :, 2:514], op=Alu.min)
                TT(out=M1[:, :, 0:512], in0=m1[:, :, 0:512],
                   in1=M1[:, :, 0:512], op=Alu.max)
                # med3(A=a1, B=M1, C=c1): u->s0, v->a1, w->c1, o->s1
                TT(out=s0[:, :, 0:512], in0=a1[:, :, 0:512],
                   in1=M1[:, :, 0:512], op=Alu.min)
                TT(out=a1[:, :, 0:512], in0=a1[:, :, 0:512],
                   in1=M1[:, :, 0:512], op=Alu.max)
                TT(out=c1[:, :, 0:512], in0=a1[:, :, 0:512],
                   in1=c1[:, :, 0:512], op=Alu.min)
                TT(out=s1[:, :, 0:512], in0=s0[:, :, 0:512],
                   in1=c1[:, :, 0:512], op=Alu.max)

                # ---- bf16 -> fp32 (scalar engine) + store on the scalar
                # HWDGE ring, two chunks so the drain overlaps.
                out_f = io.tile([128, RP, W], f32, name="out_f", tag="out_f")
                yr = yi.rearrange("(p r) w -> p r w", p=128)
                nc.scalar.copy(out=out_f[:, 0:4, :], in_=s1[:, 0:4, 0:512])
                nc.scalar.dma_start(out=yr[:, 0:4, :], in_=out_f[:, 0:4, :])
                nc.scalar.copy(out=out_f[:, 4:8, :], in_=s1[:, 4:8, 0:512])
                nc.scalar.dma_start(out=yr[:, 4:8, :], in_=out_f[:, 4:8, :])
    nc.compile()
    return nc


def _get_nc():
    if "nc" not in _STATE:
        _STATE["nc"] = _build_nc()
    return _STATE["nc"]


def kernel(x: np.ndarray) -> np.ndarray:
    from concourse.bass_utils import run_bass_kernel_spmd

    x = np.ascontiguousarray(np.asarray(x, dtype=np.float32))
    assert x.shape == (B, C, H, W), x.shape

    nc = _get_nc()
    in_maps = [
        {"x": x[i * B_LOC : (i + 1) * B_LOC].reshape(PAIRS, 2 * H, W)}
        for i in range(N_CORES)
    ]
    res = run_bass_kernel_spmd(nc, in_maps, core_ids=list(range(N_CORES)))
    _STATE["last_results"] = res
    out = np.concatenate(
        [r["y"].reshape(B_LOC, C, H, W) for r in res.results], axis=0
    )
    return out
